# revision 6
# baseline (speedup 1.0000x reference)
"""Trainium2 Bass kernel for causal self-attention (B=4, T=2048, C=1024, H=16).

Sharding: 8 cores = 4 batches (data-parallel) x 2 head-groups (tensor-parallel,
8 heads each). Each core computes QKV for its heads, flash-style causal
attention, and a partial output projection over its half of the channels;
a pairwise ReduceScatter over tokens combines the two partials per batch.

Self-contained: hardcodes shapes; host side only slices/concats numpy arrays.
"""

import numpy as np
from contextlib import ExitStack

import concourse.bass as bass
import concourse.tile as tile
from concourse import bacc, mybir
from concourse.bass_utils import run_bass_kernel_spmd
from concourse.masks import make_identity, make_upper_triangular

F32 = mybir.dt.float32
BF16 = mybir.dt.bfloat16
AF = mybir.ActivationFunctionType

B, T, C = 4, 2048, 1024
H, HD = 16, 64
G = 2                    # tensor-parallel head groups
HL = H // G              # heads per core (8)
FL = HL * HD             # local q/k/v feature width (512)
N_CORES = 8
REPLICA_GROUPS = [[2 * b, 2 * b + 1] for b in range(B)]


def _make_pools(tc, ctx):
    p = {}
    p["consts"] = ctx.enter_context(tc.tile_pool(name="consts", bufs=1))
    p["ld"] = ctx.enter_context(tc.tile_pool(name="ld", bufs=2))
    p["tsb"] = ctx.enter_context(tc.tile_pool(name="tsb", bufs=2))
    p["pp"] = ctx.enter_context(tc.tile_pool(name="pp", bufs=4))
    p["rp"] = ctx.enter_context(tc.tile_pool(name="rp", bufs=2))
    p["ft"] = ctx.enter_context(tc.tile_pool(name="ft", bufs=2))
    p["psT"] = ctx.enter_context(tc.tile_pool(name="psT", bufs=2, space="PSUM"))
    p["psMM"] = ctx.enter_context(tc.tile_pool(name="psMM", bufs=3, space="PSUM"))
    p["psY"] = ctx.enter_context(tc.tile_pool(name="psY", bufs=2, space="PSUM"))
    p["dram"] = ctx.enter_context(tc.tile_pool(name="dram", bufs=1, space="DRAM"))
    return p


def _emit_body(nc, tc, p, io, t_seq, collective=True):
    """Emit one full forward pass. t_seq: sequence length (2048, or smaller for sim)."""
    CT = C // 128          # contraction tiles (8)
    TT = t_seq // 128      # token 128-tiles
    TJ = t_seq // 512      # token 512-chunks
    FT = FL // 128         # local f 128-tiles for q/k (4)
    OT = C // 128          # output-channel tiles (8)

    consts = p["consts"]

    # ---- constants ----
    ident = consts.tile([128, 128], F32, tag="ident")
    make_identity(nc, ident)
    trimask = consts.tile([128, 128], BF16, tag="trimask")
    make_upper_triangular(nc, trimask, val=1.0, diag=True)
    ones1 = consts.tile([1, 128], BF16, tag="ones1")
    nc.vector.memset(ones1, 1.0)

    bq_t = consts.tile([128, FT], F32, tag="bq")
    nc.sync.dma_start(bq_t, io["bqs"].rearrange("(j p) -> p j", p=128))
    bk_t = consts.tile([128, FT], F32, tag="bk")
    nc.sync.dma_start(bk_t, io["bk"].rearrange("(j p) -> p j", p=128))
    bp_t = consts.tile([128, OT], F32, tag="bp")
    nc.sync.dma_start(bp_t, io["bph"].rearrange("(j p) -> p j", p=128))
    bv_f = consts.tile([1, FL], F32, tag="bvf")
    nc.sync.dma_start(bv_f, io["bv"].rearrange("(a f) -> a f", a=1))
    bv_b = consts.tile([1, FL], BF16, tag="bvb")
    nc.vector.tensor_copy(bv_b, bv_f)

    # ---- persistent big tiles ----
    xT = consts.tile([128, CT, t_seq], BF16, tag="xT")       # x^T  [c, t]
    wqT = consts.tile([128, CT, FL], BF16, tag="wqT")        # wq^T [c, f]
    wkT = consts.tile([128, CT, FL], BF16, tag="wkT")
    wvT = consts.tile([128, CT, FL], BF16, tag="wvT")
    wpT = consts.tile([128, FL // 128, C], BF16, tag="wpT")  # wp^T [c_loc, o]
    QT = consts.tile([128, FT, t_seq], BF16, tag="QT")       # q^T/8 [f, t]
    KT = consts.tile([128, FT, t_seq], BF16, tag="KT")
    V = consts.tile([128, TT, HL * 65], BF16, tag="V")       # v-aug [t, h*65+d], col 64 of each head = 1
    yT = consts.tile([128, FT, t_seq], BF16, tag="yT")       # y^T normalized [c_loc, t]

    # ones columns of V
    nc.vector.memset(V.rearrange("p t (h e) -> p t h e", h=HL)[:, :, :, 64:65], 1.0)

    # ---- phase A: load + transpose inputs ----
    def load_transpose(dram_ap, n_rowtiles, n_coltiles, dst, dst_coloff, ld_tag):
        # dram [n_rowtiles*128, n_coltiles*128] -> dst[:, ct, dst_coloff + rt*128 ...]
        for rt in range(n_rowtiles):
            ldt = p["ld"].tile([128, n_coltiles * 128], F32, tag=ld_tag)
            nc.sync.dma_start(ldt, dram_ap[rt * 128:(rt + 1) * 128, :])
            for ct in range(n_coltiles):
                ps = p["psT"].tile([128, 128], F32, tag="psT")
                nc.tensor.transpose(ps, ldt[:, ct * 128:(ct + 1) * 128], ident)
                nc.vector.tensor_copy(
                    dst[:, ct, dst_coloff + rt * 128: dst_coloff + (rt + 1) * 128], ps
                )

    load_transpose(io["wq"], FL // 128, CT, wqT, 0, "ldw")
    load_transpose(io["wk"], FL // 128, CT, wkT, 0, "ldw")
    load_transpose(io["wv"], FL // 128, CT, wvT, 0, "ldw")
    load_transpose(io["wp"], C // 128, FL // 128, wpT, 0, "ldp")
    load_transpose(io["x"], TT, CT, xT, 0, "ldw")

    # ---- phase B: QKV projection ----
    # Q^T[f, t] = sum_c wqT[c, f] * xT[c, t]   (then (·+bq)/8, bf16)
    for ft in range(FT):
        for tj in range(TJ):
            ps = p["psMM"].tile([128, 512], F32, tag="psMM")
            for ct in range(CT):
                nc.tensor.matmul(
                    ps,
                    lhsT=wqT[:, ct, ft * 128:(ft + 1) * 128],
                    rhs=xT[:, ct, tj * 512:(tj + 1) * 512],
                    start=(ct == 0),
                    stop=(ct == CT - 1),
                )
            nc.scalar.activation(
                QT[:, ft, tj * 512:(tj + 1) * 512], ps, AF.Identity,
                bias=bq_t[:, ft:ft + 1], scale=0.125,
            )
    for ft in range(FT):
        for tj in range(TJ):
            ps = p["psMM"].tile([128, 512], F32, tag="psMM")
            for ct in range(CT):
                nc.tensor.matmul(
                    ps,
                    lhsT=wkT[:, ct, ft * 128:(ft + 1) * 128],
                    rhs=xT[:, ct, tj * 512:(tj + 1) * 512],
                    start=(ct == 0),
                    stop=(ct == CT - 1),
                )
            nc.scalar.activation(
                KT[:, ft, tj * 512:(tj + 1) * 512], ps, AF.Identity,
                bias=bk_t[:, ft:ft + 1],
            )
    # V[t, f] = sum_c xT[c, t] * wvT[c, f] + bv  (bias via K=1 ones matmul)
    for tt in range(TT):
        ps = p["psMM"].tile([128, 512], F32, tag="psMM")
        for ct in range(CT):
            nc.tensor.matmul(
                ps,
                lhsT=xT[:, ct, tt * 128:(tt + 1) * 128],
                rhs=wvT[:, ct, :],
                start=(ct == 0),
                stop=False,
            )
        nc.tensor.matmul(ps, lhsT=ones1, rhs=bv_b, start=False, stop=True)
        nc.scalar.activation(
            V.rearrange("p t (h e) -> p t h e", h=HL)[:, tt, :, 0:64], ps, AF.Copy,
        )

    # ---- phase C: attention (per head, flash over k-tiles) ----
    for h in range(HL):
        ftq = h // 2
        po = (h % 2) * 64
        for jq in range(TJ):
            yp = p["psY"].tile([65, 512], F32, tag="psY")
            ni = 4 * (jq + 1)
            for i in range(ni):
                q0 = max(jq * 512, i * 128)
                qoff = q0 - jq * 512
                sp = p["psMM"].tile([128, 512], F32, tag="psMM")
                nc.tensor.matmul(
                    sp[:, qoff:512],
                    lhsT=KT[po:po + 64, ftq, i * 128:(i + 1) * 128],
                    rhs=QT[po:po + 64, ftq, q0:(jq + 1) * 512],
                    start=True,
                    stop=True,
                )
                pt = p["pp"].tile([128, 512], BF16, tag="pp")
                nc.scalar.activation(pt[:, qoff:512], sp[:, qoff:512], AF.Exp)
                if i * 128 >= jq * 512:  # diagonal block: causal mask (q >= k)
                    nc.vector.tensor_mul(
                        pt[:, qoff:qoff + 128], pt[:, qoff:qoff + 128], trimask
                    )
                nc.tensor.matmul(
                    yp[:, qoff:512],
                    lhsT=V[:, i, h * 65:h * 65 + 65],
                    rhs=pt[:, qoff:512],
                    start=(i == 0),
                    stop=(i == ni - 1),
                )
            # normalize: yT = yp[0:64] / yp[64]
            r = p["rp"].tile([1, 512], F32, tag="r")
            nc.vector.reciprocal(r, yp[64:65, :])
            R = p["rp"].tile([64, 512], F32, tag="R")
            nc.gpsimd.partition_broadcast(R, r)
            nc.vector.tensor_mul(
                yT[po:po + 64, ftq, jq * 512:(jq + 1) * 512], yp[0:64, :], R
            )

    # ---- phase D: output projection (partial over local channels) + transpose ----
    cc_in = p["dram"].tile([t_seq, C], F32, tag="cc_in")
    for tj in range(TJ):
        fts = [
            p["ft"].tile([128, C], F32, tag=f"ft{s}", name=f"ft{s}") for s in range(4)
        ]
        for ot in range(OT):
            ps = p["psMM"].tile([128, 512], F32, tag="psMM")
            for ci in range(FL // 128):
                nc.tensor.matmul(
                    ps,
                    lhsT=wpT[:, ci, ot * 128:(ot + 1) * 128],
                    rhs=yT[:, ci, tj * 512:(tj + 1) * 512],
                    start=(ci == 0),
                    stop=(ci == FL // 128 - 1),
                )
            tsb = p["tsb"].tile([128, 512], F32, tag="tsb")
            nc.scalar.activation(tsb, ps, AF.Identity, bias=bp_t[:, ot:ot + 1])
            for s in range(4):
                ps2 = p["psT"].tile([128, 128], F32, tag="psT")
                nc.tensor.transpose(ps2, tsb[:, s * 128:(s + 1) * 128], ident)
                nc.vector.tensor_copy(fts[s][:, ot * 128:(ot + 1) * 128], ps2)
        for s in range(4):
            row = (tj * 4 + s) * 128
            nc.sync.dma_start(cc_in[row:row + 128, :], fts[s])

    # ---- phase E: pairwise ReduceScatter over tokens + final store ----
    if collective:
        cc_out = p["dram"].tile([t_seq // 2, C], F32, tag="cc_out")
        nc.gpsimd.collective_compute(
            "ReduceScatter",
            mybir.AluOpType.add,
            replica_groups=REPLICA_GROUPS,
            ins=[cc_in[:].opt()],
            outs=[cc_out[:].opt()],
        )
        nc.gpsimd.dma_start(io["out"], cc_out[:])
    else:
        nc.gpsimd.dma_start(io["out"], cc_in[0:t_seq // 2, :])


def build_program(t_seq=T, repeat=1, collective=True):
    nc = bacc.Bacc("TRN2", target_bir_lowering=False, debug=False, num_devices=N_CORES)
    io = {
        "x": nc.dram_tensor("x", [t_seq, C], F32, kind="ExternalInput").ap(),
        "wq": nc.dram_tensor("wq", [FL, C], F32, kind="ExternalInput").ap(),
        "wk": nc.dram_tensor("wk", [FL, C], F32, kind="ExternalInput").ap(),
        "wv": nc.dram_tensor("wv", [FL, C], F32, kind="ExternalInput").ap(),
        "wp": nc.dram_tensor("wp", [C, FL], F32, kind="ExternalInput").ap(),
        "bqs": nc.dram_tensor("bqs", [FL], F32, kind="ExternalInput").ap(),
        "bk": nc.dram_tensor("bk", [FL], F32, kind="ExternalInput").ap(),
        "bv": nc.dram_tensor("bv", [FL], F32, kind="ExternalInput").ap(),
        "bph": nc.dram_tensor("bph", [C], F32, kind="ExternalInput").ap(),
        "out": nc.dram_tensor("out", [t_seq // 2, C], F32, kind="ExternalOutput").ap(),
    }
    with tile.TileContext(nc) as tc:
        with ExitStack() as ctx:
            pools = _make_pools(tc, ctx)
            if repeat == 1:
                _emit_body(nc, tc, pools, io, t_seq, collective=collective)
            else:
                with tc.For_i(0, repeat, 1) as _:
                    _emit_body(nc, tc, pools, io, t_seq, collective=collective)
    nc.compile()
    return nc


def make_in_maps(x, w_attn, b_attn, w_proj, b_proj):
    x = np.ascontiguousarray(np.asarray(x, dtype=np.float32))
    w_attn = np.asarray(w_attn, dtype=np.float32)
    b_attn = np.asarray(b_attn, dtype=np.float32)
    w_proj = np.asarray(w_proj, dtype=np.float32)
    b_proj = np.asarray(b_proj, dtype=np.float32)
    in_maps = []
    for c in range(N_CORES):
        b, g = c // 2, c % 2
        fs = slice(g * FL, (g + 1) * FL)
        in_maps.append({
            "x": x[b],
            "wq": np.ascontiguousarray(w_attn[0 * C:][:C][fs]),
            "wk": np.ascontiguousarray(w_attn[1 * C:][:C][fs]),
            "wv": np.ascontiguousarray(w_attn[2 * C:][:C][fs]),
            "wp": np.ascontiguousarray(w_proj[:, fs]),
            "bqs": np.ascontiguousarray(b_attn[0 * C:][:C][fs]) * np.float32(0.125),
            "bk": np.ascontiguousarray(b_attn[1 * C:][:C][fs]),
            "bv": np.ascontiguousarray(b_attn[2 * C:][:C][fs]),
            "bph": b_proj * np.float32(0.5),
        })
    return in_maps


_PROG = None


def kernel(x, w_attn, b_attn, w_proj, b_proj):
    global _PROG
    if _PROG is None:
        _PROG = build_program()
    in_maps = make_in_maps(x, w_attn, b_attn, w_proj, b_proj)
    res = run_bass_kernel_spmd(_PROG, in_maps, core_ids=list(range(N_CORES))).results
    out = np.empty((B, T, C), dtype=np.float32)
    for c in range(N_CORES):
        b, r = c // 2, c % 2
        out[b, r * (T // 2):(r + 1) * (T // 2), :] = res[c]["out"]
    return out


# revision 16
# speedup vs baseline: 1.3461x; 1.3461x over previous
"""Trainium2 Bass kernel for causal self-attention (B=4, T=2048, C=1024, H=16).

Sharding: 8 cores = 4 batches (data-parallel) x 2 head-groups (tensor-parallel,
8 heads each). Each core computes QKV for its heads, flash-style causal
attention, and a partial output projection over its half of the channels;
a pairwise ReduceScatter over tokens combines the two partials per batch.

Self-contained: hardcodes shapes; host side only slices/concats numpy arrays.
"""

import numpy as np
from contextlib import ExitStack

import concourse.bass as bass
import concourse.tile as tile
from concourse import bacc, mybir
from concourse.bass_utils import run_bass_kernel_spmd
from concourse.masks import make_identity, make_upper_triangular

F32 = mybir.dt.float32
BF16 = mybir.dt.bfloat16
AF = mybir.ActivationFunctionType
ALU = mybir.AluOpType

B, T, C = 4, 2048, 1024
H, HD = 16, 64
G = 2                    # tensor-parallel head groups
HL = H // G              # heads per core (8)
FL = HL * HD             # local q/k/v feature width (512)
N_CORES = 8
REPLICA_GROUPS = [[2 * b, 2 * b + 1] for b in range(B)]


def _make_pools(tc, ctx):
    p = {}
    p["consts"] = ctx.enter_context(tc.tile_pool(name="consts", bufs=1))
    p["ld"] = ctx.enter_context(tc.tile_pool(name="ld", bufs=3))
    p["tsb"] = ctx.enter_context(tc.tile_pool(name="tsb", bufs=2))
    p["pp"] = ctx.enter_context(tc.tile_pool(name="pp", bufs=3))
    p["rp"] = ctx.enter_context(tc.tile_pool(name="rp", bufs=2))
    p["ft"] = ctx.enter_context(tc.tile_pool(name="ft", bufs=4))
    p["psT"] = ctx.enter_context(tc.tile_pool(name="psT", bufs=2, space="PSUM"))
    p["psMM"] = ctx.enter_context(tc.tile_pool(name="psMM", bufs=2, space="PSUM"))
    p["psY"] = ctx.enter_context(tc.tile_pool(name="psY", bufs=2, space="PSUM"))
    p["dram"] = ctx.enter_context(tc.tile_pool(name="dram", bufs=1, space="DRAM"))
    return p


def _segments(qoff, width):
    """Split [qoff, width) at multiples of 512 (PSUM bank boundary)."""
    segs = []
    a = qoff
    while a < width:
        b = min((a // 512 + 1) * 512, width)
        segs.append((a, b))
        a = b
    return segs


def _emit_body(nc, tc, p, io, t_seq, collective=True):
    """Emit one full forward pass. t_seq: sequence length (2048, or smaller for sim)."""
    CT = C // 128          # contraction tiles (8)
    TT = t_seq // 128      # token 128-tiles
    TJ = t_seq // 1024     # token 1024-chunks
    FT = FL // 128         # local f 128-tiles for q/k (4)
    OT = C // 128          # output-channel tiles (8)
    PT = FL // 128         # wp c_loc tiles (4)

    consts = p["consts"]

    # ---- constants ----
    ident_f = consts.tile([128, 128], F32, tag="identf")
    make_identity(nc, ident_f)
    trimask = consts.tile([128, 128], BF16, tag="trimask")
    make_upper_triangular(nc, trimask, val=1.0, diag=True)
    ones1 = consts.tile([1, 128], BF16, tag="ones1")
    nc.vector.memset(ones1, 1.0)

    bq_t = consts.tile([128, FT], F32, tag="bq")
    nc.sync.dma_start(bq_t, io["bqs"].rearrange("(j p) -> p j", p=128))
    bk_t = consts.tile([128, FT], F32, tag="bk")
    nc.sync.dma_start(bk_t, io["bk"].rearrange("(j p) -> p j", p=128))
    bp_t = consts.tile([128, OT], F32, tag="bp")
    nc.sync.dma_start(bp_t, io["bph"].rearrange("(j p) -> p j", p=128))
    bv_f = consts.tile([1, FL], F32, tag="bvf")
    nc.sync.dma_start(bv_f, io["bv"].rearrange("(a f) -> a f", a=1))
    bv_b = consts.tile([1, FL], BF16, tag="bvb")
    nc.vector.tensor_copy(bv_b, bv_f)

    # ---- persistent tiles (split for fine-grained deps) ----
    xT = [consts.tile([128, t_seq], BF16, tag=f"xT{i}", name=f"xT{i}") for i in range(CT)]
    wqT = consts.tile([128, CT, FL], BF16, tag="wqT")
    wkT = consts.tile([128, CT, FL], BF16, tag="wkT")
    wvT = consts.tile([128, CT, FL], BF16, tag="wvT")
    wpT = consts.tile([128, PT, C], BF16, tag="wpT")
    QT = [consts.tile([128, t_seq], BF16, tag=f"QT{i}", name=f"QT{i}") for i in range(FT)]
    KT = [consts.tile([128, t_seq], BF16, tag=f"KT{i}", name=f"KT{i}") for i in range(FT)]
    Vt = [consts.tile([128, HL * 65], BF16, tag=f"Vt{i}", name=f"Vt{i}") for i in range(TT)]
    yT = [consts.tile([128, FT, 1024], BF16, tag=f"yT{i}", name=f"yT{i}") for i in range(TJ)]

    for tt in range(TT):  # ones columns of V
        nc.vector.memset(Vt[tt].rearrange("p (h e) -> p h e", h=HL)[:, :, 64:65], 1.0)

    # ---- phase A: cast-load (SWDGE f32->bf16) + PE transpose inputs ----
    def load_transpose(dram_ap, n_rowtiles, n_coltiles, dst_fn, ld_tag):
        for rt in range(n_rowtiles):
            ldt = p["ld"].tile([128, n_coltiles * 128], F32, tag=ld_tag, name=ld_tag)
            nc.sync.dma_start(ldt, dram_ap[rt * 128:(rt + 1) * 128, :])
            for ct in range(n_coltiles):
                ps = p["psT"].tile([128, 128], F32, tag="psTf", name="psTf")
                nc.tensor.transpose(ps, ldt[:, ct * 128:(ct + 1) * 128], ident_f)
                nc.vector.tensor_copy(dst_fn(ct, rt), ps)

    load_transpose(io["wq"], FT, CT,
                   lambda ct, rt: wqT[:, ct, rt * 128:(rt + 1) * 128], "ldw")
    load_transpose(io["wk"], FT, CT,
                   lambda ct, rt: wkT[:, ct, rt * 128:(rt + 1) * 128], "ldw")
    load_transpose(io["wv"], FT, CT,
                   lambda ct, rt: wvT[:, ct, rt * 128:(rt + 1) * 128], "ldw")
    load_transpose(io["wp"], OT, PT,
                   lambda ct, rt: wpT[:, ct, rt * 128:(rt + 1) * 128], "ldp")
    load_transpose(io["x"], TT, CT,
                   lambda ct, rt: xT[ct][:, rt * 128:(rt + 1) * 128], "ldw")

    # ---- phase B: QKV projection ----
    # Q^T[f, t] = (sum_c wqT[c, f] xT[c, t] + bq) / 8; K^T likewise (no scale)
    for ft in range(FT):
        for tj in range(TJ):
            ps = p["psMM"].tile([128, 1024], F32, tag="psMM", name="psMM")
            for half in range(2):
                for ct in range(CT):
                    nc.tensor.matmul(
                        ps[:, half * 512:(half + 1) * 512],
                        lhsT=wqT[:, ct, ft * 128:(ft + 1) * 128],
                        rhs=xT[ct][:, tj * 1024 + half * 512: tj * 1024 + (half + 1) * 512],
                        start=(ct == 0),
                        stop=(ct == CT - 1),
                    )
            nc.vector.tensor_scalar(
                QT[ft][:, tj * 1024:(tj + 1) * 1024], ps,
                scalar1=bq_t[:, ft:ft + 1], scalar2=0.125, op0=ALU.add, op1=ALU.mult,
            )
    for ft in range(FT):
        for tj in range(TJ):
            ps = p["psMM"].tile([128, 1024], F32, tag="psMM", name="psMM")
            for half in range(2):
                for ct in range(CT):
                    nc.tensor.matmul(
                        ps[:, half * 512:(half + 1) * 512],
                        lhsT=wkT[:, ct, ft * 128:(ft + 1) * 128],
                        rhs=xT[ct][:, tj * 1024 + half * 512: tj * 1024 + (half + 1) * 512],
                        start=(ct == 0),
                        stop=(ct == CT - 1),
                    )
            nc.vector.tensor_scalar(
                KT[ft][:, tj * 1024:(tj + 1) * 1024], ps,
                scalar1=bk_t[:, ft:ft + 1], scalar2=None, op0=ALU.add,
            )
    # V[t, f] = sum_c xT[c, t] wvT[c, f] + bv (bias via K=1 ones matmul)
    for tt in range(TT):
        ps = p["psMM"].tile([128, 1024], F32, tag="psMM", name="psMM")
        for ct in range(CT):
            nc.tensor.matmul(
                ps[:, 0:512],
                lhsT=xT[ct][:, tt * 128:(tt + 1) * 128],
                rhs=wvT[:, ct, :],
                start=(ct == 0),
                stop=False,
            )
        nc.tensor.matmul(ps[:, 0:512], lhsT=ones1, rhs=bv_b, start=False, stop=True)
        nc.vector.tensor_copy(
            Vt[tt].rearrange("p (h e) -> p h e", h=HL)[:, :, 0:64], ps[:, 0:512]
        )

    # ---- phase C: attention (chunk-outer so proj can overlap) ----
    for jq in range(TJ):
        for h in range(HL):
            ftq = h // 2
            po = (h % 2) * 64
            # two independent 512-wide Y accumulators (1 PSUM bank each)
            ypA = p["psY"].tile([65, 512], F32, tag="psY", name="psYA")
            ypB = p["psY"].tile([65, 512], F32, tag="psY", name="psYB")
            ni = 8 * (jq + 1)
            last_a = 8 * jq + 3        # last k-tile writing cols [0,512)
            for i in range(ni):
                q0 = max(jq * 1024, i * 128)
                qoff = q0 - jq * 1024
                sp = p["psMM"].tile([128, 1024], F32, tag="psMM", name="psMM")
                for (a, b) in _segments(qoff, 1024):
                    nc.tensor.matmul(
                        sp[:, a:b],
                        lhsT=KT[ftq][po:po + 64, i * 128:(i + 1) * 128],
                        rhs=QT[ftq][po:po + 64, jq * 1024 + a: jq * 1024 + b],
                        start=True,
                        stop=True,
                    )
                pt = p["pp"].tile([128, 1024], BF16, tag="pp", name="pp")
                nc.scalar.activation(pt[:, qoff:1024], sp[:, qoff:1024], AF.Exp)
                if i * 128 >= jq * 1024:  # diagonal block: causal mask (q >= k)
                    nc.vector.tensor_mul(
                        pt[:, qoff:qoff + 128], pt[:, qoff:qoff + 128], trimask
                    )
                for (a, b) in _segments(qoff, 1024):
                    yp, off, lst = (ypA, 0, last_a) if a < 512 else (ypB, 512, ni - 1)
                    nc.tensor.matmul(
                        yp[:, a - off:b - off],
                        lhsT=Vt[i][:, h * 65:h * 65 + 65],
                        rhs=pt[:, a:b],
                        start=(i == 0),
                        stop=(i == lst),
                    )
            # normalize: yT = yp[0:64] / yp[64]
            for yp, off in ((ypA, 0), (ypB, 512)):
                r = p["rp"].tile([1, 512], F32, tag="r", name="r")
                nc.vector.reciprocal(r, yp[64:65, :])
                R = p["rp"].tile([64, 512], F32, tag="R", name="R")
                nc.gpsimd.partition_broadcast(R, r)
                nc.vector.tensor_mul(
                    yT[jq][po:po + 64, ftq, off:off + 512], yp[0:64, :], R
                )

    # ---- phase D: output projection (partial over local channels) + transpose ----
    cc_in = p["dram"].tile([t_seq, C], F32, tag="cc_in")
    for tj in range(TJ):
        for ot in range(OT):
            ps = p["psMM"].tile([128, 1024], F32, tag="psMM", name="psMM")
            for half in range(2):
                for ci in range(PT):
                    nc.tensor.matmul(
                        ps[:, half * 512:(half + 1) * 512],
                        lhsT=wpT[:, ci, ot * 128:(ot + 1) * 128],
                        rhs=yT[tj][:, ci, half * 512:(half + 1) * 512],
                        start=(ci == 0),
                        stop=(ci == PT - 1),
                    )
            tsb = p["tsb"].tile([128, 1024], F32, tag="tsb", name="tsb")
            nc.vector.tensor_scalar(tsb, ps, scalar1=bp_t[:, ot:ot + 1], scalar2=None, op0=ALU.add)
            for s in range(8):
                ps2 = p["psT"].tile([128, 128], F32, tag="psTf", name="psTf")
                nc.tensor.transpose(ps2, tsb[:, s * 128:(s + 1) * 128], ident_f)
                blk = p["ft"].tile([128, 128], F32, tag="blk", name="blk")
                nc.vector.tensor_copy(blk, ps2)
                row = (tj * 8 + s) * 128
                nc.sync.dma_start(
                    cc_in[row:row + 128, ot * 128:(ot + 1) * 128], blk
                )

    # ---- phase E: pairwise ReduceScatter over tokens + final store ----
    if collective:
        cc_out = p["dram"].tile([t_seq // 2, C], F32, tag="cc_out")
        nc.gpsimd.collective_compute(
            "ReduceScatter",
            ALU.add,
            replica_groups=REPLICA_GROUPS,
            ins=[cc_in[:].opt()],
            outs=[cc_out[:].opt()],
        )
        nc.gpsimd.dma_start(io["out"], cc_out[:])
    else:
        nc.gpsimd.dma_start(io["out"], cc_in[0:t_seq // 2, :])


def build_program(t_seq=T, repeat=1, collective=True):
    nc = bacc.Bacc("TRN2", target_bir_lowering=False, debug=False, num_devices=N_CORES)
    io = {
        "x": nc.dram_tensor("x", [t_seq, C], F32, kind="ExternalInput").ap(),
        "wq": nc.dram_tensor("wq", [FL, C], F32, kind="ExternalInput").ap(),
        "wk": nc.dram_tensor("wk", [FL, C], F32, kind="ExternalInput").ap(),
        "wv": nc.dram_tensor("wv", [FL, C], F32, kind="ExternalInput").ap(),
        "wp": nc.dram_tensor("wp", [C, FL], F32, kind="ExternalInput").ap(),
        "bqs": nc.dram_tensor("bqs", [FL], F32, kind="ExternalInput").ap(),
        "bk": nc.dram_tensor("bk", [FL], F32, kind="ExternalInput").ap(),
        "bv": nc.dram_tensor("bv", [FL], F32, kind="ExternalInput").ap(),
        "bph": nc.dram_tensor("bph", [C], F32, kind="ExternalInput").ap(),
        "out": nc.dram_tensor("out", [t_seq // 2, C], F32, kind="ExternalOutput").ap(),
    }
    with tile.TileContext(nc) as tc:
        with ExitStack() as ctx:
            pools = _make_pools(tc, ctx)
            if repeat == 1:
                _emit_body(nc, tc, pools, io, t_seq, collective=collective)
            else:
                with tc.For_i(0, repeat, 1) as _:
                    _emit_body(nc, tc, pools, io, t_seq, collective=collective)
    nc.compile()
    return nc


def make_in_maps(x, w_attn, b_attn, w_proj, b_proj):
    x = np.ascontiguousarray(np.asarray(x, dtype=np.float32))
    w_attn = np.asarray(w_attn, dtype=np.float32)
    b_attn = np.asarray(b_attn, dtype=np.float32)
    w_proj = np.asarray(w_proj, dtype=np.float32)
    b_proj = np.asarray(b_proj, dtype=np.float32)
    in_maps = []
    for c in range(N_CORES):
        b, g = c // 2, c % 2
        fs = slice(g * FL, (g + 1) * FL)
        in_maps.append({
            "x": x[b],
            "wq": np.ascontiguousarray(w_attn[0 * C:][:C][fs]),
            "wk": np.ascontiguousarray(w_attn[1 * C:][:C][fs]),
            "wv": np.ascontiguousarray(w_attn[2 * C:][:C][fs]),
            "wp": np.ascontiguousarray(w_proj[:, fs]),
            "bqs": np.ascontiguousarray(b_attn[0 * C:][:C][fs]),
            "bk": np.ascontiguousarray(b_attn[1 * C:][:C][fs]),
            "bv": np.ascontiguousarray(b_attn[2 * C:][:C][fs]),
            "bph": b_proj * np.float32(0.5),
        })
    return in_maps


_PROG = None


def kernel(x, w_attn, b_attn, w_proj, b_proj):
    global _PROG
    if _PROG is None:
        _PROG = build_program()
    in_maps = make_in_maps(x, w_attn, b_attn, w_proj, b_proj)
    res = run_bass_kernel_spmd(_PROG, in_maps, core_ids=list(range(N_CORES))).results
    out = np.empty((B, T, C), dtype=np.float32)
    for c in range(N_CORES):
        b, r = c // 2, c % 2
        out[b, r * (T // 2):(r + 1) * (T // 2), :] = res[c]["out"]
    return out


# revision 18
# speedup vs baseline: 1.4502x; 1.0773x over previous
"""Trainium2 Bass kernel for causal self-attention (B=4, T=2048, C=1024, H=16).

Sharding: 8 cores = 4 batches (data-parallel) x 2 head-groups (tensor-parallel,
8 heads each). Each core computes QKV for its heads, flash-style causal
attention, and a partial output projection over its half of the channels;
a pairwise ReduceScatter over tokens combines the two partials per batch.

Self-contained: hardcodes shapes; host side only slices/concats numpy arrays.
"""

import ml_dtypes
import numpy as np
from contextlib import ExitStack

import concourse.bass as bass
import concourse.tile as tile
from concourse import bacc, mybir
from concourse.bass_utils import run_bass_kernel_spmd
from concourse.masks import make_identity, make_upper_triangular

F32 = mybir.dt.float32
BF16 = mybir.dt.bfloat16
AF = mybir.ActivationFunctionType
ALU = mybir.AluOpType

B, T, C = 4, 2048, 1024
H, HD = 16, 64
G = 2                    # tensor-parallel head groups
HL = H // G              # heads per core (8)
FL = HL * HD             # local q/k/v feature width (512)
N_CORES = 8
REPLICA_GROUPS = [[2 * b, 2 * b + 1] for b in range(B)]


def _make_pools(tc, ctx):
    p = {}
    p["consts"] = ctx.enter_context(tc.tile_pool(name="consts", bufs=1))
    p["ld"] = ctx.enter_context(tc.tile_pool(name="ld", bufs=3))
    p["tsb"] = ctx.enter_context(tc.tile_pool(name="tsb", bufs=2))
    p["pp"] = ctx.enter_context(tc.tile_pool(name="pp", bufs=3))
    p["rp"] = ctx.enter_context(tc.tile_pool(name="rp", bufs=2))
    p["ft"] = ctx.enter_context(tc.tile_pool(name="ft", bufs=4))
    p["psT"] = ctx.enter_context(tc.tile_pool(name="psT", bufs=2, space="PSUM"))
    p["psMM"] = ctx.enter_context(tc.tile_pool(name="psMM", bufs=2, space="PSUM"))
    p["psY"] = ctx.enter_context(tc.tile_pool(name="psY", bufs=2, space="PSUM"))
    p["dram"] = ctx.enter_context(tc.tile_pool(name="dram", bufs=1, space="DRAM"))
    return p


def _segments(qoff, width):
    """Split [qoff, width) at multiples of 512 (PSUM bank boundary)."""
    segs = []
    a = qoff
    while a < width:
        b = min((a // 512 + 1) * 512, width)
        segs.append((a, b))
        a = b
    return segs


def _emit_body(nc, tc, p, io, t_seq, collective=True):
    """Emit one full forward pass. t_seq: sequence length (2048, or smaller for sim)."""
    CT = C // 128          # contraction tiles (8)
    TT = t_seq // 128      # token 128-tiles
    TJ = t_seq // 1024     # token 1024-chunks
    FT = FL // 128         # local f 128-tiles for q/k (4)
    OT = C // 128          # output-channel tiles (8)
    PT = FL // 128         # wp c_loc tiles (4)

    consts = p["consts"]

    # ---- constants ----
    ident_f = consts.tile([128, 128], F32, tag="identf")
    make_identity(nc, ident_f)
    trimask = consts.tile([128, 128], BF16, tag="trimask")
    make_upper_triangular(nc, trimask, val=1.0, diag=True)
    ones1 = consts.tile([1, 128], BF16, tag="ones1")
    nc.vector.memset(ones1, 1.0)

    bq_t = consts.tile([128, FT], F32, tag="bq")
    nc.sync.dma_start(bq_t, io["bqs"].rearrange("(j p) -> p j", p=128))
    bk_t = consts.tile([128, FT], F32, tag="bk")
    nc.sync.dma_start(bk_t, io["bk"].rearrange("(j p) -> p j", p=128))
    bp_t = consts.tile([128, OT], F32, tag="bp")
    nc.sync.dma_start(bp_t, io["bph"].rearrange("(j p) -> p j", p=128))
    bv_f = consts.tile([1, FL], F32, tag="bvf")
    nc.sync.dma_start(bv_f, io["bv"].rearrange("(a f) -> a f", a=1))
    bv_b = consts.tile([1, FL], BF16, tag="bvb")
    nc.vector.tensor_copy(bv_b, bv_f)

    # ---- persistent tiles (split for fine-grained deps) ----
    xT = [consts.tile([128, t_seq], BF16, tag=f"xT{i}", name=f"xT{i}") for i in range(CT)]
    wqT = consts.tile([128, CT, FL], BF16, tag="wqT")
    wkT = consts.tile([128, CT, FL], BF16, tag="wkT")
    wvT = consts.tile([128, CT, FL], BF16, tag="wvT")
    wpT = consts.tile([128, PT, C], BF16, tag="wpT")
    QT = [consts.tile([128, t_seq], BF16, tag=f"QT{i}", name=f"QT{i}") for i in range(FT)]
    KT = [consts.tile([128, t_seq], BF16, tag=f"KT{i}", name=f"KT{i}") for i in range(FT)]
    Vt = [consts.tile([128, HL * 65], BF16, tag=f"Vt{i}", name=f"Vt{i}") for i in range(TT)]
    yT = [consts.tile([128, FT, 1024], BF16, tag=f"yT{i}", name=f"yT{i}") for i in range(TJ)]

    for tt in range(TT):  # ones columns of V
        nc.vector.memset(Vt[tt].rearrange("p (h e) -> p h e", h=HL)[:, :, 64:65], 1.0)

    # ---- phase A: inputs arrive bf16; hardware DMA-transpose straight to SBUF ----
    for ct in range(CT):
        nc.sync.dma_start(xT[ct], io["x"][:, ct * 128:(ct + 1) * 128], transpose=True)
    for ct in range(CT):
        nc.sync.dma_start(wqT[:, ct, :], io["wq"][:, ct * 128:(ct + 1) * 128], transpose=True)
        nc.sync.dma_start(wkT[:, ct, :], io["wk"][:, ct * 128:(ct + 1) * 128], transpose=True)
        nc.sync.dma_start(wvT[:, ct, :], io["wv"][:, ct * 128:(ct + 1) * 128], transpose=True)
    for ci in range(PT):
        nc.sync.dma_start(wpT[:, ci, :], io["wp"][:, ci * 128:(ci + 1) * 128], transpose=True)

    # ---- phase B: QKV projection ----
    # Q^T[f, t] = (sum_c wqT[c, f] xT[c, t] + bq) / 8; K^T likewise (no scale)
    for ft in range(FT):
        for tj in range(TJ):
            ps = p["psMM"].tile([128, 1024], F32, tag="psMM", name="psMM")
            for half in range(2):
                for ct in range(CT):
                    nc.tensor.matmul(
                        ps[:, half * 512:(half + 1) * 512],
                        lhsT=wqT[:, ct, ft * 128:(ft + 1) * 128],
                        rhs=xT[ct][:, tj * 1024 + half * 512: tj * 1024 + (half + 1) * 512],
                        start=(ct == 0),
                        stop=(ct == CT - 1),
                    )
            nc.vector.tensor_scalar(
                QT[ft][:, tj * 1024:(tj + 1) * 1024], ps,
                scalar1=bq_t[:, ft:ft + 1], scalar2=0.125, op0=ALU.add, op1=ALU.mult,
            )
    for ft in range(FT):
        for tj in range(TJ):
            ps = p["psMM"].tile([128, 1024], F32, tag="psMM", name="psMM")
            for half in range(2):
                for ct in range(CT):
                    nc.tensor.matmul(
                        ps[:, half * 512:(half + 1) * 512],
                        lhsT=wkT[:, ct, ft * 128:(ft + 1) * 128],
                        rhs=xT[ct][:, tj * 1024 + half * 512: tj * 1024 + (half + 1) * 512],
                        start=(ct == 0),
                        stop=(ct == CT - 1),
                    )
            nc.vector.tensor_scalar(
                KT[ft][:, tj * 1024:(tj + 1) * 1024], ps,
                scalar1=bk_t[:, ft:ft + 1], scalar2=None, op0=ALU.add,
            )
    # V[t, f] = sum_c xT[c, t] wvT[c, f] + bv (bias via K=1 ones matmul)
    for tt in range(TT):
        ps = p["psMM"].tile([128, 1024], F32, tag="psMM", name="psMM")
        for ct in range(CT):
            nc.tensor.matmul(
                ps[:, 0:512],
                lhsT=xT[ct][:, tt * 128:(tt + 1) * 128],
                rhs=wvT[:, ct, :],
                start=(ct == 0),
                stop=False,
            )
        nc.tensor.matmul(ps[:, 0:512], lhsT=ones1, rhs=bv_b, start=False, stop=True)
        nc.vector.tensor_copy(
            Vt[tt].rearrange("p (h e) -> p h e", h=HL)[:, :, 0:64], ps[:, 0:512]
        )

    # ---- phase C: attention (chunk-outer so proj can overlap) ----
    for jq in range(TJ):
        for h in range(HL):
            ftq = h // 2
            po = (h % 2) * 64
            # two independent 512-wide Y accumulators (1 PSUM bank each)
            ypA = p["psY"].tile([65, 512], F32, tag="psY", name="psYA")
            ypB = p["psY"].tile([65, 512], F32, tag="psY", name="psYB")
            ni = 8 * (jq + 1)
            last_a = 8 * jq + 3        # last k-tile writing cols [0,512)
            for i in range(ni):
                q0 = max(jq * 1024, i * 128)
                qoff = q0 - jq * 1024
                sp = p["psMM"].tile([128, 1024], F32, tag="psMM", name="psMM")
                for (a, b) in _segments(qoff, 1024):
                    nc.tensor.matmul(
                        sp[:, a:b],
                        lhsT=KT[ftq][po:po + 64, i * 128:(i + 1) * 128],
                        rhs=QT[ftq][po:po + 64, jq * 1024 + a: jq * 1024 + b],
                        start=True,
                        stop=True,
                    )
                pt = p["pp"].tile([128, 1024], BF16, tag="pp", name="pp")
                nc.scalar.activation(pt[:, qoff:1024], sp[:, qoff:1024], AF.Exp)
                if i * 128 >= jq * 1024:  # diagonal block: causal mask (q >= k)
                    nc.vector.tensor_mul(
                        pt[:, qoff:qoff + 128], pt[:, qoff:qoff + 128], trimask
                    )
                for (a, b) in _segments(qoff, 1024):
                    yp, off, lst = (ypA, 0, last_a) if a < 512 else (ypB, 512, ni - 1)
                    nc.tensor.matmul(
                        yp[:, a - off:b - off],
                        lhsT=Vt[i][:, h * 65:h * 65 + 65],
                        rhs=pt[:, a:b],
                        start=(i == 0),
                        stop=(i == lst),
                    )
            # normalize: yT = yp[0:64] / yp[64]
            for yp, off in ((ypA, 0), (ypB, 512)):
                r = p["rp"].tile([1, 512], F32, tag="r", name="r")
                nc.vector.reciprocal(r, yp[64:65, :])
                R = p["rp"].tile([64, 512], F32, tag="R", name="R")
                nc.gpsimd.partition_broadcast(R, r)
                nc.vector.tensor_mul(
                    yT[jq][po:po + 64, ftq, off:off + 512], yp[0:64, :], R
                )

    # ---- phase D: output projection (partial over local channels) + transpose ----
    cc_in = p["dram"].tile([t_seq, C], F32, tag="cc_in")
    for tj in range(TJ):
        for ot in range(OT):
            ps = p["psMM"].tile([128, 1024], F32, tag="psMM", name="psMM")
            for half in range(2):
                for ci in range(PT):
                    nc.tensor.matmul(
                        ps[:, half * 512:(half + 1) * 512],
                        lhsT=wpT[:, ci, ot * 128:(ot + 1) * 128],
                        rhs=yT[tj][:, ci, half * 512:(half + 1) * 512],
                        start=(ci == 0),
                        stop=(ci == PT - 1),
                    )
            tsb = p["tsb"].tile([128, 1024], F32, tag="tsb", name="tsb")
            nc.vector.tensor_scalar(tsb, ps, scalar1=bp_t[:, ot:ot + 1], scalar2=None, op0=ALU.add)
            for s in range(8):
                ps2 = p["psT"].tile([128, 128], F32, tag="psTf", name="psTf")
                nc.tensor.transpose(ps2, tsb[:, s * 128:(s + 1) * 128], ident_f)
                blk = p["ft"].tile([128, 128], F32, tag="blk", name="blk")
                nc.vector.tensor_copy(blk, ps2)
                row = (tj * 8 + s) * 128
                nc.sync.dma_start(
                    cc_in[row:row + 128, ot * 128:(ot + 1) * 128], blk
                )

    # ---- phase E: pairwise ReduceScatter over tokens + final store ----
    if collective:
        cc_out = p["dram"].tile([t_seq // 2, C], F32, tag="cc_out")
        nc.gpsimd.collective_compute(
            "ReduceScatter",
            ALU.add,
            replica_groups=REPLICA_GROUPS,
            ins=[cc_in[:].opt()],
            outs=[cc_out[:].opt()],
        )
        nc.gpsimd.dma_start(io["out"], cc_out[:])
    else:
        nc.gpsimd.dma_start(io["out"], cc_in[0:t_seq // 2, :])


def build_program(t_seq=T, repeat=1, collective=True):
    nc = bacc.Bacc("TRN2", target_bir_lowering=False, debug=False, num_devices=N_CORES)
    io = {
        "x": nc.dram_tensor("x", [t_seq, C], BF16, kind="ExternalInput").ap(),
        "wq": nc.dram_tensor("wq", [FL, C], BF16, kind="ExternalInput").ap(),
        "wk": nc.dram_tensor("wk", [FL, C], BF16, kind="ExternalInput").ap(),
        "wv": nc.dram_tensor("wv", [FL, C], BF16, kind="ExternalInput").ap(),
        "wp": nc.dram_tensor("wp", [C, FL], BF16, kind="ExternalInput").ap(),
        "bqs": nc.dram_tensor("bqs", [FL], F32, kind="ExternalInput").ap(),
        "bk": nc.dram_tensor("bk", [FL], F32, kind="ExternalInput").ap(),
        "bv": nc.dram_tensor("bv", [FL], F32, kind="ExternalInput").ap(),
        "bph": nc.dram_tensor("bph", [C], F32, kind="ExternalInput").ap(),
        "out": nc.dram_tensor("out", [t_seq // 2, C], F32, kind="ExternalOutput").ap(),
    }
    with tile.TileContext(nc) as tc:
        with ExitStack() as ctx:
            pools = _make_pools(tc, ctx)
            if repeat == 1:
                _emit_body(nc, tc, pools, io, t_seq, collective=collective)
            else:
                with tc.For_i(0, repeat, 1) as _:
                    _emit_body(nc, tc, pools, io, t_seq, collective=collective)
    nc.compile()
    return nc


def make_in_maps(x, w_attn, b_attn, w_proj, b_proj):
    x = np.ascontiguousarray(np.asarray(x, dtype=np.float32))
    w_attn = np.asarray(w_attn, dtype=np.float32)
    b_attn = np.asarray(b_attn, dtype=np.float32)
    w_proj = np.asarray(w_proj, dtype=np.float32)
    b_proj = np.asarray(b_proj, dtype=np.float32)
    in_maps = []
    for c in range(N_CORES):
        b, g = c // 2, c % 2
        fs = slice(g * FL, (g + 1) * FL)
        bf = ml_dtypes.bfloat16
        in_maps.append({
            "x": x[b].astype(bf),
            "wq": np.ascontiguousarray(w_attn[0 * C:][:C][fs]).astype(bf),
            "wk": np.ascontiguousarray(w_attn[1 * C:][:C][fs]).astype(bf),
            "wv": np.ascontiguousarray(w_attn[2 * C:][:C][fs]).astype(bf),
            "wp": np.ascontiguousarray(w_proj[:, fs]).astype(bf),
            "bqs": np.ascontiguousarray(b_attn[0 * C:][:C][fs]),
            "bk": np.ascontiguousarray(b_attn[1 * C:][:C][fs]),
            "bv": np.ascontiguousarray(b_attn[2 * C:][:C][fs]),
            "bph": b_proj * np.float32(0.5),
        })
    return in_maps


_PROG = None


def kernel(x, w_attn, b_attn, w_proj, b_proj):
    global _PROG
    if _PROG is None:
        _PROG = build_program()
    in_maps = make_in_maps(x, w_attn, b_attn, w_proj, b_proj)
    res = run_bass_kernel_spmd(_PROG, in_maps, core_ids=list(range(N_CORES))).results
    out = np.empty((B, T, C), dtype=np.float32)
    for c in range(N_CORES):
        b, r = c // 2, c % 2
        out[b, r * (T // 2):(r + 1) * (T // 2), :] = res[c]["out"]
    return out


# revision 19
# speedup vs baseline: 1.4878x; 1.0260x over previous
"""Trainium2 Bass kernel for causal self-attention (B=4, T=2048, C=1024, H=16).

Sharding: 8 cores = 4 batches (data-parallel) x 2 head-groups (tensor-parallel,
8 heads each). Each core computes QKV for its heads, flash-style causal
attention, and a partial output projection over its half of the channels;
a pairwise ReduceScatter over tokens combines the two partials per batch.

Self-contained: hardcodes shapes; host side only slices/concats numpy arrays.
"""

import ml_dtypes
import numpy as np
from contextlib import ExitStack

import concourse.bass as bass
import concourse.tile as tile
from concourse import bacc, mybir
from concourse.bass_utils import run_bass_kernel_spmd
from concourse.masks import make_identity, make_upper_triangular

F32 = mybir.dt.float32
BF16 = mybir.dt.bfloat16
AF = mybir.ActivationFunctionType
ALU = mybir.AluOpType

B, T, C = 4, 2048, 1024
H, HD = 16, 64
G = 2                    # tensor-parallel head groups
HL = H // G              # heads per core (8)
FL = HL * HD             # local q/k/v feature width (512)
N_CORES = 8
REPLICA_GROUPS = [[2 * b, 2 * b + 1] for b in range(B)]


def _make_pools(tc, ctx):
    p = {}
    p["consts"] = ctx.enter_context(tc.tile_pool(name="consts", bufs=1))
    p["tsb"] = ctx.enter_context(tc.tile_pool(name="tsb", bufs=2))
    p["pp"] = ctx.enter_context(tc.tile_pool(name="pp", bufs=4))
    p["rp"] = ctx.enter_context(tc.tile_pool(name="rp", bufs=2))
    p["ft"] = ctx.enter_context(tc.tile_pool(name="ft", bufs=4))
    p["psMM"] = ctx.enter_context(tc.tile_pool(name="psMM", bufs=3, space="PSUM"))
    p["psY"] = ctx.enter_context(tc.tile_pool(name="psY", bufs=2, space="PSUM"))
    p["dram"] = ctx.enter_context(tc.tile_pool(name="dram", bufs=1, space="DRAM"))
    return p


def _segments(qoff, width):
    """Split [qoff, width) at multiples of 512 (PSUM bank boundary)."""
    segs = []
    a = qoff
    while a < width:
        b = min((a // 512 + 1) * 512, width)
        segs.append((a, b))
        a = b
    return segs


def _emit_body(nc, tc, p, io, t_seq, collective=True):
    """Emit one full forward pass. t_seq: sequence length (2048, or smaller for sim)."""
    CT = C // 128          # contraction tiles (8)
    TT = t_seq // 128      # token 128-tiles
    TJ = t_seq // 1024     # token 1024-chunks
    FT = FL // 128         # local f 128-tiles for q/k (4)
    OT = C // 128          # output-channel tiles (8)
    PT = FL // 128         # wp c_loc tiles (4)

    consts = p["consts"]

    # ---- constants ----
    ident_f = consts.tile([128, 128], F32, tag="identf")
    make_identity(nc, ident_f)
    trimask = consts.tile([128, 128], BF16, tag="trimask")
    make_upper_triangular(nc, trimask, val=1.0, diag=True)
    ones1 = consts.tile([1, 128], BF16, tag="ones1")
    nc.vector.memset(ones1, 1.0)

    bq_t = consts.tile([128, FT], F32, tag="bq")
    nc.sync.dma_start(bq_t, io["bqs"].rearrange("(j p) -> p j", p=128))
    bk_t = consts.tile([128, FT], F32, tag="bk")
    nc.sync.dma_start(bk_t, io["bk"].rearrange("(j p) -> p j", p=128))
    bp_t = consts.tile([128, OT], F32, tag="bp")
    nc.sync.dma_start(bp_t, io["bph"].rearrange("(j p) -> p j", p=128))
    bv_f = consts.tile([1, FL], F32, tag="bvf")
    nc.sync.dma_start(bv_f, io["bv"].rearrange("(a f) -> a f", a=1))
    bv_b = consts.tile([1, FL], BF16, tag="bvb")
    nc.vector.tensor_copy(bv_b, bv_f)

    # ---- persistent tiles (split for fine-grained deps) ----
    xT = [consts.tile([128, t_seq], BF16, tag=f"xT{i}", name=f"xT{i}") for i in range(CT)]
    wqT = consts.tile([128, CT, FL], BF16, tag="wqT")
    wkT = consts.tile([128, CT, FL], BF16, tag="wkT")
    wvT = consts.tile([128, CT, FL], BF16, tag="wvT")
    wpT = consts.tile([128, PT, C], BF16, tag="wpT")
    QT = [consts.tile([128, t_seq], BF16, tag=f"QT{i}", name=f"QT{i}") for i in range(FT)]
    KT = [consts.tile([128, t_seq], BF16, tag=f"KT{i}", name=f"KT{i}") for i in range(FT)]
    Vt = [consts.tile([128, HL * 65], BF16, tag=f"Vt{i}", name=f"Vt{i}") for i in range(TT)]
    yT = [consts.tile([128, FT, 1024], BF16, tag=f"yT{i}", name=f"yT{i}") for i in range(TJ)]

    for tt in range(TT):  # ones columns of V
        nc.vector.memset(Vt[tt].rearrange("p (h e) -> p h e", h=HL)[:, :, 64:65], 1.0)

    # ---- phase A: inputs arrive bf16; hardware DMA-transpose straight to SBUF ----
    for ct in range(CT):
        nc.sync.dma_start(xT[ct], io["x"][:, ct * 128:(ct + 1) * 128], transpose=True)
    for ct in range(CT):
        nc.sync.dma_start(wqT[:, ct, :], io["wq"][:, ct * 128:(ct + 1) * 128], transpose=True)
        nc.sync.dma_start(wkT[:, ct, :], io["wk"][:, ct * 128:(ct + 1) * 128], transpose=True)
        nc.sync.dma_start(wvT[:, ct, :], io["wv"][:, ct * 128:(ct + 1) * 128], transpose=True)
    for ci in range(PT):
        nc.sync.dma_start(wpT[:, ci, :], io["wp"][:, ci * 128:(ci + 1) * 128], transpose=True)

    # ---- phase B: QKV projection ----
    # Q^T[f, t] = (sum_c wqT[c, f] xT[c, t] + bq) / 8; K^T likewise (no scale)
    for ft in range(FT):
        for tj in range(TJ):
            ps = p["psMM"].tile([128, 1024], F32, tag="psMM", name="psMM")
            for half in range(2):
                for ct in range(CT):
                    nc.tensor.matmul(
                        ps[:, half * 512:(half + 1) * 512],
                        lhsT=wqT[:, ct, ft * 128:(ft + 1) * 128],
                        rhs=xT[ct][:, tj * 1024 + half * 512: tj * 1024 + (half + 1) * 512],
                        start=(ct == 0),
                        stop=(ct == CT - 1),
                    )
            nc.vector.tensor_scalar(
                QT[ft][:, tj * 1024:(tj + 1) * 1024], ps,
                scalar1=bq_t[:, ft:ft + 1], scalar2=0.125, op0=ALU.add, op1=ALU.mult,
            )
    for ft in range(FT):
        for tj in range(TJ):
            ps = p["psMM"].tile([128, 1024], F32, tag="psMM", name="psMM")
            for half in range(2):
                for ct in range(CT):
                    nc.tensor.matmul(
                        ps[:, half * 512:(half + 1) * 512],
                        lhsT=wkT[:, ct, ft * 128:(ft + 1) * 128],
                        rhs=xT[ct][:, tj * 1024 + half * 512: tj * 1024 + (half + 1) * 512],
                        start=(ct == 0),
                        stop=(ct == CT - 1),
                    )
            nc.vector.tensor_scalar(
                KT[ft][:, tj * 1024:(tj + 1) * 1024], ps,
                scalar1=bk_t[:, ft:ft + 1], scalar2=None, op0=ALU.add,
            )
    # V[t, f] = sum_c xT[c, t] wvT[c, f] + bv (bias via K=1 ones matmul)
    for tt in range(TT):
        ps = p["psMM"].tile([128, 1024], F32, tag="psMM", name="psMM")
        for ct in range(CT):
            nc.tensor.matmul(
                ps[:, 0:512],
                lhsT=xT[ct][:, tt * 128:(tt + 1) * 128],
                rhs=wvT[:, ct, :],
                start=(ct == 0),
                stop=False,
            )
        nc.tensor.matmul(ps[:, 0:512], lhsT=ones1, rhs=bv_b, start=False, stop=True)
        nc.vector.tensor_copy(
            Vt[tt].rearrange("p (h e) -> p h e", h=HL)[:, :, 0:64], ps[:, 0:512]
        )

    # ---- phase C: attention (chunk-outer so proj can overlap) ----
    for jq in range(TJ):
        for h in range(HL):
            ftq = h // 2
            po = (h % 2) * 64
            # two independent 512-wide Y accumulators (1 PSUM bank each)
            ypA = p["psY"].tile([65, 512], F32, tag="psY", name="psYA")
            ypB = p["psY"].tile([65, 512], F32, tag="psY", name="psYB")
            ni = 8 * (jq + 1)
            last_a = 8 * jq + 3        # last k-tile writing cols [0,512)
            for i in range(ni):
                q0 = max(jq * 1024, i * 128)
                qoff = q0 - jq * 1024
                sp = p["psMM"].tile([128, 1024], F32, tag="psMM", name="psMM")
                for (a, b) in _segments(qoff, 1024):
                    nc.tensor.matmul(
                        sp[:, a:b],
                        lhsT=KT[ftq][po:po + 64, i * 128:(i + 1) * 128],
                        rhs=QT[ftq][po:po + 64, jq * 1024 + a: jq * 1024 + b],
                        start=True,
                        stop=True,
                    )
                pt = p["pp"].tile([128, 1024], BF16, tag="pp", name="pp")
                nc.scalar.activation(pt[:, qoff:1024], sp[:, qoff:1024], AF.Exp)
                if i * 128 >= jq * 1024:  # diagonal block: causal mask (q >= k)
                    nc.vector.tensor_mul(
                        pt[:, qoff:qoff + 128], pt[:, qoff:qoff + 128], trimask
                    )
                for (a, b) in _segments(qoff, 1024):
                    yp, off, lst = (ypA, 0, last_a) if a < 512 else (ypB, 512, ni - 1)
                    nc.tensor.matmul(
                        yp[:, a - off:b - off],
                        lhsT=Vt[i][:, h * 65:h * 65 + 65],
                        rhs=pt[:, a:b],
                        start=(i == 0),
                        stop=(i == lst),
                    )
            # normalize: yT = yp[0:64] / yp[64]
            for yp, off in ((ypA, 0), (ypB, 512)):
                r = p["rp"].tile([1, 512], F32, tag="r", name="r")
                nc.vector.reciprocal(r, yp[64:65, :])
                R = p["rp"].tile([64, 512], F32, tag="R", name="R")
                nc.gpsimd.partition_broadcast(R, r)
                nc.vector.tensor_mul(
                    yT[jq][po:po + 64, ftq, off:off + 512], yp[0:64, :], R
                )

    # ---- phase D: output projection (partial over local channels) + transpose ----
    cc_in = p["dram"].tile([t_seq, C], F32, tag="cc_in")
    for tj in range(TJ):
        for ot in range(OT):
            ps = p["psMM"].tile([128, 1024], F32, tag="psMM", name="psMM")
            for half in range(2):
                for ci in range(PT):
                    nc.tensor.matmul(
                        ps[:, half * 512:(half + 1) * 512],
                        lhsT=wpT[:, ci, ot * 128:(ot + 1) * 128],
                        rhs=yT[tj][:, ci, half * 512:(half + 1) * 512],
                        start=(ci == 0),
                        stop=(ci == PT - 1),
                    )
            tsb = p["tsb"].tile([128, 1024], F32, tag="tsb", name="tsb")
            nc.vector.tensor_scalar(tsb, ps, scalar1=bp_t[:, ot:ot + 1], scalar2=None, op0=ALU.add)
            for s in range(8):
                ps2 = p["psMM"].tile([128, 128], F32, tag="psMM", name="psTf")
                nc.tensor.transpose(ps2, tsb[:, s * 128:(s + 1) * 128], ident_f)
                blk = p["ft"].tile([128, 128], F32, tag="blk", name="blk")
                nc.vector.tensor_copy(blk, ps2)
                row = (tj * 8 + s) * 128
                nc.sync.dma_start(
                    cc_in[row:row + 128, ot * 128:(ot + 1) * 128], blk
                )

    # ---- phase E: pairwise ReduceScatter over tokens + final store ----
    if collective:
        cc_out = p["dram"].tile([t_seq // 2, C], F32, tag="cc_out")
        nc.gpsimd.collective_compute(
            "ReduceScatter",
            ALU.add,
            replica_groups=REPLICA_GROUPS,
            ins=[cc_in[:].opt()],
            outs=[cc_out[:].opt()],
        )
        nc.gpsimd.dma_start(io["out"], cc_out[:])
    else:
        nc.gpsimd.dma_start(io["out"], cc_in[0:t_seq // 2, :])


def build_program(t_seq=T, repeat=1, collective=True):
    nc = bacc.Bacc("TRN2", target_bir_lowering=False, debug=False, num_devices=N_CORES)
    io = {
        "x": nc.dram_tensor("x", [t_seq, C], BF16, kind="ExternalInput").ap(),
        "wq": nc.dram_tensor("wq", [FL, C], BF16, kind="ExternalInput").ap(),
        "wk": nc.dram_tensor("wk", [FL, C], BF16, kind="ExternalInput").ap(),
        "wv": nc.dram_tensor("wv", [FL, C], BF16, kind="ExternalInput").ap(),
        "wp": nc.dram_tensor("wp", [C, FL], BF16, kind="ExternalInput").ap(),
        "bqs": nc.dram_tensor("bqs", [FL], F32, kind="ExternalInput").ap(),
        "bk": nc.dram_tensor("bk", [FL], F32, kind="ExternalInput").ap(),
        "bv": nc.dram_tensor("bv", [FL], F32, kind="ExternalInput").ap(),
        "bph": nc.dram_tensor("bph", [C], F32, kind="ExternalInput").ap(),
        "out": nc.dram_tensor("out", [t_seq // 2, C], F32, kind="ExternalOutput").ap(),
    }
    with tile.TileContext(nc) as tc:
        with ExitStack() as ctx:
            pools = _make_pools(tc, ctx)
            if repeat == 1:
                _emit_body(nc, tc, pools, io, t_seq, collective=collective)
            else:
                with tc.For_i(0, repeat, 1) as _:
                    _emit_body(nc, tc, pools, io, t_seq, collective=collective)
    nc.compile()
    return nc


def make_in_maps(x, w_attn, b_attn, w_proj, b_proj):
    x = np.ascontiguousarray(np.asarray(x, dtype=np.float32))
    w_attn = np.asarray(w_attn, dtype=np.float32)
    b_attn = np.asarray(b_attn, dtype=np.float32)
    w_proj = np.asarray(w_proj, dtype=np.float32)
    b_proj = np.asarray(b_proj, dtype=np.float32)
    in_maps = []
    for c in range(N_CORES):
        b, g = c // 2, c % 2
        fs = slice(g * FL, (g + 1) * FL)
        bf = ml_dtypes.bfloat16
        in_maps.append({
            "x": x[b].astype(bf),
            "wq": np.ascontiguousarray(w_attn[0 * C:][:C][fs]).astype(bf),
            "wk": np.ascontiguousarray(w_attn[1 * C:][:C][fs]).astype(bf),
            "wv": np.ascontiguousarray(w_attn[2 * C:][:C][fs]).astype(bf),
            "wp": np.ascontiguousarray(w_proj[:, fs]).astype(bf),
            "bqs": np.ascontiguousarray(b_attn[0 * C:][:C][fs]),
            "bk": np.ascontiguousarray(b_attn[1 * C:][:C][fs]),
            "bv": np.ascontiguousarray(b_attn[2 * C:][:C][fs]),
            "bph": b_proj * np.float32(0.5),
        })
    return in_maps


_PROG = None


def kernel(x, w_attn, b_attn, w_proj, b_proj):
    global _PROG
    if _PROG is None:
        _PROG = build_program()
    in_maps = make_in_maps(x, w_attn, b_attn, w_proj, b_proj)
    res = run_bass_kernel_spmd(_PROG, in_maps, core_ids=list(range(N_CORES))).results
    out = np.empty((B, T, C), dtype=np.float32)
    for c in range(N_CORES):
        b, r = c // 2, c % 2
        out[b, r * (T // 2):(r + 1) * (T // 2), :] = res[c]["out"]
    return out


# revision 28
# speedup vs baseline: 1.8903x; 1.2705x over previous
"""Trainium2 Bass kernel for causal self-attention (B=4, T=2048, C=1024, H=16).

Sharding: 8 cores = 4 batches (data-parallel) x 2 head-groups (tensor-parallel,
8 heads each). Each core computes QKV for its heads, flash-style causal
attention, and a partial output projection over its half of the channels;
a pairwise ReduceScatter over tokens combines the two partials per batch.

Self-contained: hardcodes shapes; host side only slices/concats numpy arrays.
"""

import ml_dtypes
import numpy as np
from contextlib import ExitStack

import concourse.bass as bass
import concourse.tile as tile
from concourse import bacc, mybir
from concourse.bass_utils import run_bass_kernel_spmd
from concourse.masks import make_identity, make_upper_triangular

F32 = mybir.dt.float32
BF16 = mybir.dt.bfloat16
AF = mybir.ActivationFunctionType
ALU = mybir.AluOpType

B, T, C = 4, 2048, 1024
H, HD = 16, 64
G = 2                    # tensor-parallel head groups
HL = H // G              # heads per core (8)
FL = HL * HD             # local q/k/v feature width (512)
N_CORES = 8
REPLICA_GROUPS = [[2 * b, 2 * b + 1] for b in range(B)]


def _make_pools(tc, ctx):
    p = {}
    p["consts"] = ctx.enter_context(tc.tile_pool(name="consts", bufs=1))
    p["tsb"] = ctx.enter_context(tc.tile_pool(name="tsb", bufs=2))
    p["pp"] = ctx.enter_context(tc.tile_pool(name="pp", bufs=4))
    p["rp"] = ctx.enter_context(tc.tile_pool(name="rp", bufs=4))
    p["ft"] = ctx.enter_context(tc.tile_pool(name="ft", bufs=4))
    p["psMM"] = ctx.enter_context(tc.tile_pool(name="psMM", bufs=2, space="PSUM"))
    p["psY"] = ctx.enter_context(tc.tile_pool(name="psY", bufs=4, space="PSUM"))
    p["dram"] = ctx.enter_context(tc.tile_pool(name="dram", bufs=1, space="DRAM"))
    return p


def _segments(qoff, width):
    """Split [qoff, width) at multiples of 512 (PSUM bank boundary)."""
    segs = []
    a = qoff
    while a < width:
        b = min((a // 512 + 1) * 512, width)
        segs.append((a, b))
        a = b
    return segs


def cc_stub(p, t_seq):
    return p["dram"].tile([t_seq // 2, C], F32, tag="cc_stub", name="cc_stub")[:]


def _emit_body(nc, tc, p, io, t_seq, collective=True, upto='E'):
    """Emit one full forward pass. t_seq: sequence length (2048, or smaller for sim)."""
    CT = C // 128          # contraction tiles (8)
    TT = t_seq // 128      # token 128-tiles
    TJ = t_seq // 1024     # token 1024-chunks
    FT = FL // 128         # local f 128-tiles for q/k (4)
    OT = C // 128          # output-channel tiles (8)
    PT = FL // 128         # wp c_loc tiles (4)

    consts = p["consts"]

    # ---- constants ----
    ident_f = consts.tile([128, 128], F32, tag="identf")
    make_identity(nc, ident_f)
    ident_b = consts.tile([128, 128], BF16, tag="identb")
    make_identity(nc, ident_b)
    negtri = consts.tile([128, 128], BF16, tag="negtri")
    make_upper_triangular(nc, negtri, val=-50.0, diag=False)
    ones1 = consts.tile([1, 128], BF16, tag="ones1")
    nc.vector.memset(ones1, 1.0)

    bq_t = consts.tile([128, FT], F32, tag="bq")
    nc.sync.dma_start(bq_t, io["bqs"].rearrange("(j p) -> p j", p=128))
    bk_t = consts.tile([128, FT], F32, tag="bk")
    nc.sync.dma_start(bk_t, io["bk"].rearrange("(j p) -> p j", p=128))
    bp_t = consts.tile([128, OT], F32, tag="bp")
    nc.sync.dma_start(bp_t, io["bph"].rearrange("(j p) -> p j", p=128))
    bv_f = consts.tile([1, FL], F32, tag="bvf")
    nc.sync.dma_start(bv_f, io["bv"].rearrange("(a f) -> a f", a=1))
    bv_b = consts.tile([1, FL], BF16, tag="bvb")
    nc.vector.tensor_copy(bv_b, bv_f)

    # ---- persistent tiles (split for fine-grained deps) ----
    xT = [consts.tile([128, t_seq], BF16, tag=f"xT{i}", name=f"xT{i}") for i in range(CT)]
    wqT = consts.tile([128, CT, FL], BF16, tag="wqT")
    wkT = consts.tile([128, CT, FL], BF16, tag="wkT")
    wvT = consts.tile([128, CT, FL], BF16, tag="wvT")
    wpT = consts.tile([128, PT, C], BF16, tag="wpT")
    QT = [consts.tile([128, t_seq], BF16, tag=f"QT{i}", name=f"QT{i}") for i in range(FT)]
    KT = [consts.tile([128, t_seq], BF16, tag=f"KT{i}", name=f"KT{i}") for i in range(FT)]
    Vt = [consts.tile([128, HL * 65], BF16, tag=f"Vt{i}", name=f"Vt{i}") for i in range(TT)]
    yT = [consts.tile([128, FT, 1024], BF16, tag=f"yT{i}", name=f"yT{i}") for i in range(TJ)]

    for tt in range(TT):  # ones columns of V
        nc.vector.memset(Vt[tt].rearrange("p (h e) -> p h e", h=HL)[:, :, 64:65], 1.0)

    # ---- phase A: inputs arrive bf16; hardware DMA-transpose straight to SBUF ----
    for ct in range(CT):
        nc.sync.dma_start(xT[ct], io["x"][:, ct * 128:(ct + 1) * 128], transpose=True)
    for ct in range(CT):
        nc.sync.dma_start(wqT[:, ct, :], io["wq"][:, ct * 128:(ct + 1) * 128], transpose=True)
        nc.sync.dma_start(wkT[:, ct, :], io["wk"][:, ct * 128:(ct + 1) * 128], transpose=True)
        nc.sync.dma_start(wvT[:, ct, :], io["wv"][:, ct * 128:(ct + 1) * 128], transpose=True)
    for ci in range(PT):
        nc.sync.dma_start(wpT[:, ci, :], io["wp"][:, ci * 128:(ci + 1) * 128], transpose=True)

    if upto == 'A':
        nc.gpsimd.dma_start(io["out"], cc_stub(p, t_seq))
        return
    # ---- phase B: QKV projection ----
    # Q^T[f, t] = (sum_c wqT[c, f] xT[c, t] + bq) / 8; K^T likewise (no scale)
    for dst, w_T, bias, scl in ((QT, wqT, bq_t, 0.125), (KT, wkT, bk_t, None)):
        for ft in range(FT):
            for tj in range(TJ):
                ps = p["psMM"].tile([128, 1024], F32, tag="psMM", name="psMM")
                for half in range(2):
                    for ct in range(CT):
                        nc.tensor.matmul(
                            ps[:, half * 512:(half + 1) * 512],
                            lhsT=w_T[:, ct, ft * 128:(ft + 1) * 128],
                            rhs=xT[ct][:, tj * 1024 + half * 512: tj * 1024 + (half + 1) * 512],
                            start=(ct == 0),
                            stop=(ct == CT - 1),
                        )
                if scl is None:
                    nc.vector.tensor_scalar(
                        dst[ft][:, tj * 1024:(tj + 1) * 1024], ps,
                        scalar1=bias[:, ft:ft + 1], scalar2=None, op0=ALU.add,
                    )
                else:
                    nc.vector.tensor_scalar(
                        dst[ft][:, tj * 1024:(tj + 1) * 1024], ps,
                        scalar1=bias[:, ft:ft + 1], scalar2=scl,
                        op0=ALU.add, op1=ALU.mult,
                    )
    # V[t, f] = sum_c xT[c, t] wvT[c, f] + bv (bias via K=1 ones matmul)
    for tt in range(TT):
        ps = p["psMM"].tile([128, 1024], F32, tag="psMM", name="psMM")
        for ct in range(CT):
            nc.tensor.matmul(
                ps[:, 0:512],
                lhsT=xT[ct][:, tt * 128:(tt + 1) * 128],
                rhs=wvT[:, ct, :],
                start=(ct == 0),
                stop=False,
            )
        nc.tensor.matmul(ps[:, 0:512], lhsT=ones1, rhs=bv_b, start=False, stop=True)
        nc.vector.tensor_copy(
            Vt[tt].rearrange("p (h e) -> p h e", h=HL)[:, :, 0:64], ps[:, 0:512]
        )

    if upto == 'B':
        nc.gpsimd.dma_start(io["out"], cc_stub(p, t_seq))
        return
    # ---- phase C: attention, 1024-wide q-chunks (chunk-outer so proj overlaps) ----
    for jq in range(TJ):
        for h in range(HL):
            ftq = h // 2
            po = (h % 2) * 64
            # two independent 512-wide Y accumulators (1 PSUM bank each)
            ypA = p["psY"].tile([65, 512], F32, tag="psY", name="psYA")
            ypB = p["psY"].tile([65, 512], F32, tag="psY", name="psYB")
            ni = 8 * (jq + 1)
            last_a = 8 * jq + 3        # last k-tile writing cols [0,512)
            for i in range(ni):
                q0 = max(jq * 1024, i * 128)
                qoff = q0 - jq * 1024
                diag = i * 128 >= jq * 1024
                sp = p["psMM"].tile([128, 1024], F32, tag="psMM", name="psMM")
                for si, (a, b) in enumerate(_segments(qoff, 1024)):
                    nc.tensor.matmul(
                        sp[:, a:b],
                        lhsT=KT[ftq][po:po + 64, i * 128:(i + 1) * 128],
                        rhs=QT[ftq][po:po + 64, jq * 1024 + a: jq * 1024 + b],
                        start=True,
                        stop=not (diag and si == 0),
                    )
                if diag:
                    # causal mask: add -50 to q<k of the diagonal 128x128 block
                    # (strict-upper^T @ I)[k, q] = [q < k]
                    nc.tensor.matmul(
                        sp[:, qoff:qoff + 128],
                        lhsT=negtri,
                        rhs=ident_b,
                        start=False,
                        stop=True,
                    )
                pt = p["pp"].tile([128, 1024], BF16, tag="pp", name="pp")
                nc.scalar.activation(pt[:, qoff:1024], sp[:, qoff:1024], AF.Exp)
                for (a, b) in _segments(qoff, 1024):
                    yp, off, lst = (ypA, 0, last_a) if a < 512 else (ypB, 512, ni - 1)
                    nc.tensor.matmul(
                        yp[:, a - off:b - off],
                        lhsT=Vt[i][:, h * 65:h * 65 + 65],
                        rhs=pt[:, a:b],
                        start=(i == 0),
                        stop=(i == lst),
                    )
            # normalize: yT = yp[0:64] / yp[64]; copy out of PSUM first so the
            # accumulator slot frees after one DVE op, not the whole chain
            for yp, off in ((ypA, 0), (ypB, 512)):
                yc = p["rp"].tile([65, 512], F32, tag="yc", name="yc")
                nc.vector.tensor_copy(yc, yp)
                r = p["rp"].tile([1, 512], F32, tag="r", name="r")
                nc.vector.reciprocal(r, yc[64:65, :])
                R = p["rp"].tile([64, 512], F32, tag="R", name="R")
                nc.gpsimd.partition_broadcast(R, r)
                nc.vector.tensor_mul(
                    yT[jq][po:po + 64, ftq, off:off + 512], yc[0:64, :], R
                )

    if upto == 'C':
        nc.gpsimd.dma_start(io["out"], cc_stub(p, t_seq))
        return
    # ---- phase D: output projection (partial over local channels) + transpose ----
    cc_in = p["dram"].tile([t_seq, C], F32, tag="cc_in")
    for tj in range(TJ):
        for ot in range(OT):
            ps = p["psMM"].tile([128, 1024], F32, tag="psMM", name="psMM")
            for half in range(2):
                for ci in range(PT):
                    nc.tensor.matmul(
                        ps[:, half * 512:(half + 1) * 512],
                        lhsT=wpT[:, ci, ot * 128:(ot + 1) * 128],
                        rhs=yT[tj][:, ci, half * 512:(half + 1) * 512],
                        start=(ci == 0),
                        stop=(ci == PT - 1),
                    )
            tsb = p["tsb"].tile([128, 1024], F32, tag="tsb", name="tsb")
            nc.vector.tensor_scalar(tsb, ps, scalar1=bp_t[:, ot:ot + 1], scalar2=None, op0=ALU.add)
            psD = p["psMM"].tile([128, 1024], F32, tag="psMM", name="psD")
            for s in range(8):
                nc.tensor.transpose(
                    psD[:, s * 128:(s + 1) * 128], tsb[:, s * 128:(s + 1) * 128], ident_f
                )
            blk = p["ft"].tile([128, 1024], F32, tag="blk", name="blk")
            nc.vector.tensor_copy(blk, psD)
            for s in range(8):
                row = (tj * 8 + s) * 128
                nc.sync.dma_start(
                    cc_in[row:row + 128, ot * 128:(ot + 1) * 128],
                    blk[:, s * 128:(s + 1) * 128],
                )

    # ---- phase E: pairwise ReduceScatter over tokens + final store ----
    if collective:
        cc_out = p["dram"].tile([t_seq // 2, C], F32, tag="cc_out")
        nc.gpsimd.collective_compute(
            "ReduceScatter",
            ALU.add,
            replica_groups=REPLICA_GROUPS,
            ins=[cc_in[:].opt()],
            outs=[cc_out[:].opt()],
        )
        nc.gpsimd.dma_start(io["out"], cc_out[:])
    else:
        nc.gpsimd.dma_start(io["out"], cc_in[0:t_seq // 2, :])


def build_program(t_seq=T, repeat=1, collective=True, upto='E'):
    nc = bacc.Bacc("TRN2", target_bir_lowering=False, debug=False, num_devices=N_CORES)
    io = {
        "x": nc.dram_tensor("x", [t_seq, C], BF16, kind="ExternalInput").ap(),
        "wq": nc.dram_tensor("wq", [FL, C], BF16, kind="ExternalInput").ap(),
        "wk": nc.dram_tensor("wk", [FL, C], BF16, kind="ExternalInput").ap(),
        "wv": nc.dram_tensor("wv", [FL, C], BF16, kind="ExternalInput").ap(),
        "wp": nc.dram_tensor("wp", [C, FL], BF16, kind="ExternalInput").ap(),
        "bqs": nc.dram_tensor("bqs", [FL], F32, kind="ExternalInput").ap(),
        "bk": nc.dram_tensor("bk", [FL], F32, kind="ExternalInput").ap(),
        "bv": nc.dram_tensor("bv", [FL], F32, kind="ExternalInput").ap(),
        "bph": nc.dram_tensor("bph", [C], F32, kind="ExternalInput").ap(),
        "out": nc.dram_tensor("out", [t_seq // 2, C], F32, kind="ExternalOutput").ap(),
    }
    with tile.TileContext(nc) as tc:
        with ExitStack() as ctx:
            pools = _make_pools(tc, ctx)
            if repeat == 1:
                _emit_body(nc, tc, pools, io, t_seq, collective=collective, upto=upto)
            else:
                with tc.For_i(0, repeat, 1) as _:
                    _emit_body(nc, tc, pools, io, t_seq, collective=collective, upto=upto)
    nc.compile()
    return nc


def make_in_maps(x, w_attn, b_attn, w_proj, b_proj):
    x = np.ascontiguousarray(np.asarray(x, dtype=np.float32))
    w_attn = np.asarray(w_attn, dtype=np.float32)
    b_attn = np.asarray(b_attn, dtype=np.float32)
    w_proj = np.asarray(w_proj, dtype=np.float32)
    b_proj = np.asarray(b_proj, dtype=np.float32)
    in_maps = []
    for c in range(N_CORES):
        b, g = c // 2, c % 2
        fs = slice(g * FL, (g + 1) * FL)
        bf = ml_dtypes.bfloat16
        in_maps.append({
            "x": x[b].astype(bf),
            "wq": np.ascontiguousarray(w_attn[0 * C:][:C][fs]).astype(bf),
            "wk": np.ascontiguousarray(w_attn[1 * C:][:C][fs]).astype(bf),
            "wv": np.ascontiguousarray(w_attn[2 * C:][:C][fs]).astype(bf),
            "wp": np.ascontiguousarray(w_proj[:, fs]).astype(bf),
            "bqs": np.ascontiguousarray(b_attn[0 * C:][:C][fs]),
            "bk": np.ascontiguousarray(b_attn[1 * C:][:C][fs]),
            "bv": np.ascontiguousarray(b_attn[2 * C:][:C][fs]),
            "bph": b_proj * np.float32(0.5),
        })
    return in_maps


_PROG = None


def kernel(x, w_attn, b_attn, w_proj, b_proj):
    global _PROG
    if _PROG is None:
        _PROG = build_program()
    in_maps = make_in_maps(x, w_attn, b_attn, w_proj, b_proj)
    res = run_bass_kernel_spmd(_PROG, in_maps, core_ids=list(range(N_CORES))).results
    out = np.empty((B, T, C), dtype=np.float32)
    for c in range(N_CORES):
        b, r = c // 2, c % 2
        out[b, r * (T // 2):(r + 1) * (T // 2), :] = res[c]["out"]
    return out
